# revision 14
# baseline (speedup 1.0000x reference)
"""CrossModalityAttention Trainium2 Bass kernel (reassociated).

Data-parallel over batch: 8 cores, one batch element each.

Algebra: with M = Wq@Wk^T (host-precomputed, batch-independent) the
attention scores are S = img@M@txt^T (contraction through HID=1024
replaced by TXT=768), and the attended output is
(softmax(S)@txt)@Wv, so the Q/K/V projections disappear entirely:
  - bk shifts every score in a row equally -> softmax-invariant, dropped
  - bq contributes a per-key additive term c*txt@(Wk@bq), folded into
    the exp as a per-partition activation bias
  - bv passes through the softmax average (weights sum to 1), added on
    the host at the end
Per-core MACs drop from 13.96G to 9.66G, and the q*k*t scores matmul
runs fp8-e4m3 DoubleRow (2x PE rate; G is scaled x16 via M to dodge
e4m3 subnormals, compensated in the exp scale).

Phases (all SBUF-resident, PSUM ring of [128,2048]f32 quads):
  GT   : GT[t,q]  = (16M)^T img^T   bf16 -> fp8 (DR pairs)   98.3k cyc
  S    : qd[k,q]  = txtT8^T GT8     fp8 DR, exp(+bias)->E    98.3k
  AV1  : W8T[t,q] = txt^T-slices E  bf16                    196.6k
  AV2  : outT[h,q]= Wv^T-slices W8T bf16, x 1/rowsum        98.3k
Row sums: ones^T acc (DVE-accumulated E) -> reciprocal, kept as a
partition-broadcast [128,2048] so the transposed output scales with a
plain tensor_tensor multiply (no DRAM bounce).
Output is produced transposed [HID, LQ]; the host transposes back.
"""

import ml_dtypes
import numpy as np

import concourse.bass as bass
import concourse.tile as tile
from concourse import bacc, mybir
from concourse.bass_utils import run_bass_kernel_spmd

F32 = mybir.dt.float32
F32R = mybir.dt.float32r
BF16 = mybir.dt.bfloat16
FP8 = mybir.dt.float8e4
AF = mybir.ActivationFunctionType
DR = mybir.MatmulPerfMode.DoubleRow

P = 128
B, LQ, LK = 8, 2048, 2048
IMG, TXT, HID = 1024, 768, 1024
NIC = IMG // P           # 8 img-dim chunks
NTC = TXT // P           # 6 txt-dim chunks
NTP = NTC // 2           # 3 DoubleRow pairs over txt dim
NKT = LK // P            # 16 key tiles
NQT = LQ // P            # 16 query tiles
NHT = HID // P           # 8 hid tiles
WS = 16.0                # weight pre-scale (power of 2, exact)
SCALE = 1.0 / np.sqrt(np.float32(HID))

_CACHED = {}


def build_kernel():
    nc = bacc.Bacc("TRN2", target_bir_lowering=False, debug=False)
    imgT = nc.dram_tensor("imgT", [IMG, LQ], BF16, kind="ExternalInput").ap()
    txt = nc.dram_tensor("txt", [LK, TXT], BF16, kind="ExternalInput").ap()
    txtT8 = nc.dram_tensor("txtT8", [NTP * P, 2 * LK], FP8,
                           kind="ExternalInput").ap()
    m16 = nc.dram_tensor("m16", [IMG, TXT], BF16, kind="ExternalInput").ap()
    wv = nc.dram_tensor("wv", [TXT, HID], BF16, kind="ExternalInput").ap()
    ctxtr = nc.dram_tensor("ctxtr", [LK], F32, kind="ExternalInput").ap()
    outT = nc.dram_tensor("outT", [HID, LQ], F32, kind="ExternalOutput").ap()

    with tile.TileContext(nc) as tc:
        with (
            tc.tile_pool(name="p0", bufs=1) as p0,
            tc.tile_pool(name="ps", bufs=1, space="PSUM") as ps,
        ):
            ones_f = p0.tile([P, P], F32, tag="ones_f")
            nc.vector.memset(ones_f[:], 1.0)
            ones_r = p0.tile([P, P], F32R, tag="ones")
            nc.vector.tensor_copy(ones_r[:], ones_f[:])
            ctx_t = p0.tile([P, NKT], F32, tag="ctx")
            nc.scalar.dma_start(out=ctx_t[:],
                                in_=ctxtr.rearrange("(j p) -> p j", p=P))

            # fp8 moving for scores: GT tiles in DoubleRow pairs
            g8 = [p0.tile([P, 2 * LQ], FP8, tag=f"g8_{i}", name=f"g8_{i}")
                  for i in range(NTP)]

            acc = p0.tile([P, LQ], F32R, tag="acc")
            recip = p0.tile([P, LQ], F32, tag="recip")

            # ---------- phase GT: GT[t,q] = (16M)^T @ img^T ----------
            # img/m tiles are dead after this phase; their buffers are
            # reused below (same tags) for e_t[0:8] / txt_r[0:8].
            # m16 lands first on the otherwise-empty gpsimd queue (needed by
            # the very first matmul); img chunks split across sync+scalar so
            # GT's c-loop never outruns the arrivals.
            m_r = []
            for c in range(NIC):
                t = p0.tile([P, TXT], BF16, tag=f"m{c}", name=f"m{c}")
                nc.gpsimd.dma_start(out=t[:], in_=m16[c * P:(c + 1) * P, :])
                m_r.append(t)
            img_r = []
            for c in range(NIC):
                t = p0.tile([P, LQ], BF16, tag=f"img{c}", name=f"img{c}")
                eng = nc.sync if c % 2 == 0 else nc.scalar
                eng.dma_start(out=t[:], in_=imgT[c * P:(c + 1) * P, :])
                img_r.append(t)
            # fp8 stationary for scores: txt^T in DoubleRow pairs (needed
            # only once the scores phase starts, ~40us in)
            t8 = [p0.tile([P, 2 * LK], FP8, tag=f"t8_{i}", name=f"t8_{i}")
                  for i in range(NTP)]
            for i in range(NTP):
                nc.gpsimd.dma_start(out=t8[i][:],
                                    in_=txtT8[i * P:(i + 1) * P, :])

            for tt in range(NTC):
                gp = ps.tile([P, LQ], F32, tag="big", bufs=2, name="gp")
                for c in range(NIC):
                    for qc in range(4):
                        nc.tensor.matmul(
                            gp[:, qc * 512:(qc + 1) * 512],
                            m_r[c][:, tt * P:(tt + 1) * P],
                            img_r[c][:, qc * 512:(qc + 1) * 512],
                            start=(c == 0),
                            stop=(c == NIC - 1),
                        )
                nc.scalar.copy(
                    g8[tt // 2][:, (tt % 2) * LQ:(tt % 2 + 1) * LQ], gp[:]
                )

            # inputs for AV1/AV2; txt_r[0:8] reuse the m-tile buffers (the
            # DMA waits for the last GT matmul read, which is long before
            # AV1 needs them)
            txt_r = []
            for k in range(NKT):
                tag = f"m{k}" if k < NIC else f"txt{k}"
                t = p0.tile([P, TXT], BF16, tag=tag, name=f"txt{k}")
                nc.gpsimd.dma_start(out=t[:], in_=txt[k * P:(k + 1) * P, :])
                txt_r.append(t)
            wv_r = []
            for c in range(NTC):
                t = p0.tile([P, HID], BF16, tag=f"wv{c}", name=f"wv{c}")
                nc.gpsimd.dma_start(out=t[:], in_=wv[c * P:(c + 1) * P, :])
                wv_r.append(t)

            # ---------- phase S: scores + exp ----------
            # e_t[0:8] reuse the img-tile buffers (dead once GT is done)
            e_t = [p0.tile([P, LQ], BF16,
                           tag=(f"img{k}" if k < NIC else f"e{k}"),
                           name=f"e{k}")
                   for k in range(NKT)]
            t8v = [t8[i][:].rearrange("p (j x) -> p j x", j=2)
                   for i in range(NTP)]
            g8v = [g8[i][:].rearrange("p (j x) -> p j x", j=2)
                   for i in range(NTP)]
            for k in range(NKT):
                qd = ps.tile([P, LQ], F32, tag="big", bufs=2, name="qd")
                for i in range(NTP):
                    for qc in range(4):
                        nc.tensor.matmul(
                            qd[:, qc * 512:(qc + 1) * 512],
                            t8v[i][:, :, k * P:(k + 1) * P],
                            g8v[i][:, :, qc * 512:(qc + 1) * 512],
                            start=(i == 0),
                            stop=(i == NTP - 1),
                            perf_mode=DR,
                        )
                nc.scalar.activation(
                    e_t[k][:], qd[:], AF.Exp,
                    scale=float(SCALE / WS), bias=ctx_t[:, k:k + 1],
                )
                if k == 0:
                    nc.vector.tensor_copy(acc[:], e_t[0][:])
                else:
                    nc.vector.tensor_add(acc[:], acc[:], e_t[k][:])

            # ---------- phase AV1: W8T[t,q] = E^T-contracted txt ----------
            w8t = [p0.tile([P, LQ], BF16, tag=f"w8_{t}", name=f"w8_{t}")
                   for t in range(NTC)]
            for tt in range(NTC):
                wp = ps.tile([P, LQ], F32, tag="big", bufs=2, name="wp")
                for k in range(NKT):
                    for qc in range(4):
                        nc.tensor.matmul(
                            wp[:, qc * 512:(qc + 1) * 512],
                            txt_r[k][:, tt * P:(tt + 1) * P],
                            e_t[k][:, qc * 512:(qc + 1) * 512],
                            start=(k == 0),
                            stop=(k == NKT - 1),
                        )
                nc.scalar.copy(w8t[tt][:], wp[:])
                if tt == 0:
                    # row sums: ones^T acc gives the per-query sum broadcast
                    # across all 128 partitions; reciprocal once, reuse for
                    # every outT tile.  Scheduled here (not right after
                    # scores) so the PE never waits on the acc chain.
                    pp = ps.tile([P, LQ], F32, tag="big", bufs=2, name="pp")
                    for qc in range(4):
                        nc.tensor.matmul(
                            pp[:, qc * 512:(qc + 1) * 512],
                            ones_r[:],
                            acc[:, qc * 512:(qc + 1) * 512],
                            start=True, stop=True,
                        )
                    nc.vector.reciprocal(recip[:], pp[:])

            # ---------- phase AV2: outT[h,q] = Wv^T W8T, scaled ----------
            for ht in range(NHT):
                # qc-outer: each 512-col chunk finishes its c-accumulation
                # before the next starts, so its scale+store overlaps the
                # remaining chunks' matmuls and the kernel tail is one small
                # chunk instead of a full 1MB tile
                po = ps.tile([P, LQ], F32, tag="big", bufs=2, name="po")
                ot = p0.tile([P, LQ], F32, tag="ot", bufs=2, name="ot")
                for qc in range(4):
                    sl = slice(qc * 512, (qc + 1) * 512)
                    for c in range(NTC):
                        nc.tensor.matmul(
                            po[:, sl],
                            wv_r[c][:, ht * P:(ht + 1) * P],
                            w8t[c][:, sl],
                            start=(c == 0),
                            stop=(c == NTC - 1),
                        )
                    nc.vector.tensor_mul(ot[:, sl], po[:, sl], recip[:, sl])
                    nc.sync.dma_start(out=outT[ht * P:(ht + 1) * P, sl],
                                      in_=ot[:, sl])

    nc.compile()
    return nc


def _get_nc():
    if "nc" not in _CACHED:
        _CACHED["nc"] = build_kernel()
    return _CACHED["nc"]


def _bf16(x):
    return np.ascontiguousarray(
        np.asarray(x, np.float32).astype(ml_dtypes.bfloat16)
    )


def make_in_maps(image_features, text_features, Wq, bq, Wk, bk, Wv, bv):
    f8 = ml_dtypes.float8_e4m3
    Wq64 = np.asarray(Wq, np.float64)
    Wk64 = np.asarray(Wk, np.float64)
    m16 = _bf16(WS * (Wq64 @ Wk64.T))
    r = (Wk64 @ np.asarray(bq, np.float64)).astype(np.float32)
    wv16 = _bf16(Wv)
    img = np.asarray(image_features, np.float32)
    txt = np.asarray(text_features, np.float32)

    maps = []
    for b in range(B):
        txt8 = txt[b].astype(f8)  # [LK, TXT], quantized once from f32
        txtT8 = np.ascontiguousarray(
            txt8.T.reshape(NTP, 2, P, LK).transpose(0, 2, 1, 3)
            .reshape(NTP * P, 2 * LK)
        )
        maps.append({
            "imgT": np.ascontiguousarray(img[b].astype(ml_dtypes.bfloat16).T),
            "txt": _bf16(txt[b]),
            "txtT8": txtT8,
            "m16": m16,
            "wv": wv16,
            "ctxtr": (SCALE * (txt[b] @ r)).astype(np.float32),
        })
    return maps


def kernel(image_features, text_features, Wq, bq, Wk, bk, Wv, bv):
    in_maps = make_in_maps(image_features, text_features, Wq, bq, Wk, bk,
                           Wv, bv)
    res = run_bass_kernel_spmd(_get_nc(), in_maps, core_ids=list(range(B)))
    bv32 = np.asarray(bv, np.float32)
    return np.stack([res.results[b]["outT"].T + bv32 for b in range(B)])


# revision 16
# speedup vs baseline: 1.0117x; 1.0117x over previous
"""CrossModalityAttention Trainium2 Bass kernel (reassociated).

Data-parallel over batch: 8 cores, one batch element each.

Algebra: with M = Wq@Wk^T (host-precomputed, batch-independent) the
attention scores are S = img@M@txt^T (contraction through HID=1024
replaced by TXT=768), and the attended output is
(softmax(S)@txt)@Wv, so the Q/K/V projections disappear entirely:
  - bk shifts every score in a row equally -> softmax-invariant, dropped
  - bq contributes a per-key additive term c*txt@(Wk@bq), folded into
    the exp as a per-partition activation bias
  - bv passes through the softmax average (weights sum to 1), added on
    the host at the end
Per-core MACs drop from 13.96G to 9.66G, and the q*k*t scores matmul
runs fp8-e4m3 DoubleRow (2x PE rate; G is scaled x16 via M to dodge
e4m3 subnormals, compensated in the exp scale).

Phases (all SBUF-resident, PSUM ring of [128,2048]f32 quads):
  GT   : GT[t,q]  = (16M)^T img^T   bf16 -> fp8 (DR pairs)   98.3k cyc
  S    : qd[k,q]  = txtT8^T GT8     fp8 DR, exp(+bias)->E    98.3k
  AV1  : W8T[t,q] = txt^T-slices E  bf16                    196.6k
  AV2  : outT[h,q]= Wv^T-slices W8T bf16, x 1/rowsum        98.3k
Row sums: ones^T acc (DVE-accumulated E) -> reciprocal, kept as a
partition-broadcast [128,2048] so the transposed output scales with a
plain tensor_tensor multiply (no DRAM bounce).
Output is produced transposed [HID, LQ]; the host transposes back.
"""

import ml_dtypes
import numpy as np

import concourse.bass as bass
import concourse.tile as tile
from concourse import bacc, mybir
from concourse.bass_utils import run_bass_kernel_spmd

F32 = mybir.dt.float32
F32R = mybir.dt.float32r
BF16 = mybir.dt.bfloat16
FP8 = mybir.dt.float8e4
AF = mybir.ActivationFunctionType
DR = mybir.MatmulPerfMode.DoubleRow

P = 128
B, LQ, LK = 8, 2048, 2048
IMG, TXT, HID = 1024, 768, 1024
NIC = IMG // P           # 8 img-dim chunks
NTC = TXT // P           # 6 txt-dim chunks
NTP = NTC // 2           # 3 DoubleRow pairs over txt dim
NKT = LK // P            # 16 key tiles
NQT = LQ // P            # 16 query tiles
NHT = HID // P           # 8 hid tiles
WS = 16.0                # weight pre-scale (power of 2, exact)
SCALE = 1.0 / np.sqrt(np.float32(HID))

_CACHED = {}


def build_kernel():
    nc = bacc.Bacc("TRN2", target_bir_lowering=False, debug=False)
    imgT = nc.dram_tensor("imgT", [IMG, LQ], BF16, kind="ExternalInput").ap()
    txt = nc.dram_tensor("txt", [LK, TXT], BF16, kind="ExternalInput").ap()
    txtT8 = nc.dram_tensor("txtT8", [NTP * P, 2 * LK], FP8,
                           kind="ExternalInput").ap()
    m16 = nc.dram_tensor("m16", [IMG, TXT], BF16, kind="ExternalInput").ap()
    wv = nc.dram_tensor("wv", [TXT, HID], BF16, kind="ExternalInput").ap()
    ctxtr = nc.dram_tensor("ctxtr", [LK], F32, kind="ExternalInput").ap()
    outT = nc.dram_tensor("outT", [HID, LQ], F32, kind="ExternalOutput").ap()

    with tile.TileContext(nc) as tc:
        with (
            tc.tile_pool(name="p0", bufs=1) as p0,
            tc.tile_pool(name="ps", bufs=1, space="PSUM") as ps,
        ):
            ones_f = p0.tile([P, P], F32, tag="ones_f")
            nc.vector.memset(ones_f[:], 1.0)
            ones_r = p0.tile([P, P], F32R, tag="ones")
            nc.vector.tensor_copy(ones_r[:], ones_f[:])
            ctx_t = p0.tile([P, NKT], F32, tag="ctx")
            nc.scalar.dma_start(out=ctx_t[:],
                                in_=ctxtr.rearrange("(j p) -> p j", p=P))

            # fp8 moving for scores: GT tiles in DoubleRow pairs
            g8 = [p0.tile([P, 2 * LQ], FP8, tag=f"g8_{i}", name=f"g8_{i}")
                  for i in range(NTP)]

            acc = p0.tile([P, LQ], F32R, tag="acc")
            recip = p0.tile([P, LQ], F32, tag="recip")

            # ---------- phase GT: GT[t,q] = (16M)^T @ img^T ----------
            # img/m tiles are dead after this phase; their buffers are
            # reused below (same tags) for e_t[0:8] / txt_r[0:8].
            # DMA packets from all queues round-robin across the shared DMA
            # engines, so whatever is issued first gets the bandwidth.  The
            # first GT matmuls need all of m16 (the c-contraction), so m16
            # goes at the head of BOTH the sync and scalar queues; the img
            # chunks follow, alternating, and are consumed slower (3.4us per
            # chunk) than they arrive.  Everything later-phase (txtT8, txt,
            # wv) streams on gpsimd behind them.
            m_r = []
            for c in range(NIC):
                t = p0.tile([P, TXT], BF16, tag=f"m{c}", name=f"m{c}")
                eng = nc.sync if c % 2 == 0 else nc.scalar
                eng.dma_start(out=t[:], in_=m16[c * P:(c + 1) * P, :])
                m_r.append(t)
            img_r = []
            for c in range(NIC):
                t = p0.tile([P, LQ], BF16, tag=f"img{c}", name=f"img{c}")
                eng = nc.sync if c % 2 == 0 else nc.scalar
                eng.dma_start(out=t[:], in_=imgT[c * P:(c + 1) * P, :])
                img_r.append(t)
            # fp8 stationary for scores: txt^T in DoubleRow pairs (needed
            # only once the scores phase starts, ~40us in)
            t8 = [p0.tile([P, 2 * LK], FP8, tag=f"t8_{i}", name=f"t8_{i}")
                  for i in range(NTP)]
            for i in range(NTP):
                nc.gpsimd.dma_start(out=t8[i][:],
                                    in_=txtT8[i * P:(i + 1) * P, :])

            for tt in range(NTC):
                gp = ps.tile([P, LQ], F32, tag="big", bufs=2, name="gp")
                for c in range(NIC):
                    for qc in range(4):
                        nc.tensor.matmul(
                            gp[:, qc * 512:(qc + 1) * 512],
                            m_r[c][:, tt * P:(tt + 1) * P],
                            img_r[c][:, qc * 512:(qc + 1) * 512],
                            start=(c == 0),
                            stop=(c == NIC - 1),
                        )
                nc.scalar.copy(
                    g8[tt // 2][:, (tt % 2) * LQ:(tt % 2 + 1) * LQ], gp[:]
                )

            # inputs for AV1/AV2; txt_r[0:8] reuse the m-tile buffers (the
            # DMA waits for the last GT matmul read, which is long before
            # AV1 needs them)
            txt_r = []
            for k in range(NKT):
                tag = f"m{k}" if k < NIC else f"txt{k}"
                t = p0.tile([P, TXT], BF16, tag=tag, name=f"txt{k}")
                nc.gpsimd.dma_start(out=t[:], in_=txt[k * P:(k + 1) * P, :])
                txt_r.append(t)
            wv_r = []
            for c in range(NTC):
                t = p0.tile([P, HID], BF16, tag=f"wv{c}", name=f"wv{c}")
                nc.gpsimd.dma_start(out=t[:], in_=wv[c * P:(c + 1) * P, :])
                wv_r.append(t)

            # ---------- phase S: scores + exp ----------
            # e_t[0:8] reuse the img-tile buffers (dead once GT is done)
            e_t = [p0.tile([P, LQ], BF16,
                           tag=(f"img{k}" if k < NIC else f"e{k}"),
                           name=f"e{k}")
                   for k in range(NKT)]
            t8v = [t8[i][:].rearrange("p (j x) -> p j x", j=2)
                   for i in range(NTP)]
            g8v = [g8[i][:].rearrange("p (j x) -> p j x", j=2)
                   for i in range(NTP)]
            for k in range(NKT):
                qd = ps.tile([P, LQ], F32, tag="big", bufs=2, name="qd")
                for i in range(NTP):
                    for qc in range(4):
                        nc.tensor.matmul(
                            qd[:, qc * 512:(qc + 1) * 512],
                            t8v[i][:, :, k * P:(k + 1) * P],
                            g8v[i][:, :, qc * 512:(qc + 1) * 512],
                            start=(i == 0),
                            stop=(i == NTP - 1),
                            perf_mode=DR,
                        )
                nc.scalar.activation(
                    e_t[k][:], qd[:], AF.Exp,
                    scale=float(SCALE / WS), bias=ctx_t[:, k:k + 1],
                )
                if k == 0:
                    nc.vector.tensor_copy(acc[:], e_t[0][:])
                else:
                    nc.vector.tensor_add(acc[:], acc[:], e_t[k][:])

            # ---------- phase AV1: W8T[t,q] = E^T-contracted txt ----------
            w8t = [p0.tile([P, LQ], BF16, tag=f"w8_{t}", name=f"w8_{t}")
                   for t in range(NTC)]
            for tt in range(NTC):
                wp = ps.tile([P, LQ], F32, tag="big", bufs=2, name="wp")
                for k in range(NKT):
                    for qc in range(4):
                        nc.tensor.matmul(
                            wp[:, qc * 512:(qc + 1) * 512],
                            txt_r[k][:, tt * P:(tt + 1) * P],
                            e_t[k][:, qc * 512:(qc + 1) * 512],
                            start=(k == 0),
                            stop=(k == NKT - 1),
                        )
                nc.scalar.copy(w8t[tt][:], wp[:])
                if tt == 0:
                    # row sums: ones^T acc gives the per-query sum broadcast
                    # across all 128 partitions; reciprocal once, reuse for
                    # every outT tile.  Scheduled here (not right after
                    # scores) so the PE never waits on the acc chain.
                    pp = ps.tile([P, LQ], F32, tag="big", bufs=2, name="pp")
                    for qc in range(4):
                        nc.tensor.matmul(
                            pp[:, qc * 512:(qc + 1) * 512],
                            ones_r[:],
                            acc[:, qc * 512:(qc + 1) * 512],
                            start=True, stop=True,
                        )
                    nc.vector.reciprocal(recip[:], pp[:])

            # ---------- phase AV2: outT[h,q] = Wv^T W8T, scaled ----------
            for ht in range(NHT):
                ot = p0.tile([P, LQ], F32, tag="ot", bufs=2, name="ot")
                if ht < NHT - 1:
                    po = ps.tile([P, LQ], F32, tag="big", bufs=2, name="po")
                    for c in range(NTC):
                        for qc in range(4):
                            nc.tensor.matmul(
                                po[:, qc * 512:(qc + 1) * 512],
                                wv_r[c][:, ht * P:(ht + 1) * P],
                                w8t[c][:, qc * 512:(qc + 1) * 512],
                                start=(c == 0),
                                stop=(c == NTC - 1),
                            )
                    nc.vector.tensor_mul(ot[:], po[:], recip[:])
                    for qc in range(4):
                        sl = slice(qc * 512, (qc + 1) * 512)
                        nc.sync.dma_start(out=outT[ht * P:(ht + 1) * P, sl],
                                          in_=ot[:, sl])
                else:
                    # last tile: pipeline per 512-col chunk through separate
                    # PSUM ring slots so the kernel tail is one small chunk's
                    # scale+store, not a full 1MB tile's
                    for qc in range(4):
                        sl = slice(qc * 512, (qc + 1) * 512)
                        poq = ps.tile([P, LQ], F32, tag="big", bufs=2,
                                      name="poq")
                        for c in range(NTC):
                            nc.tensor.matmul(
                                poq[:, 0:512],
                                wv_r[c][:, ht * P:(ht + 1) * P],
                                w8t[c][:, sl],
                                start=(c == 0),
                                stop=(c == NTC - 1),
                            )
                        nc.vector.tensor_mul(ot[:, sl], poq[:, 0:512],
                                             recip[:, sl])
                        nc.sync.dma_start(out=outT[ht * P:(ht + 1) * P, sl],
                                          in_=ot[:, sl])

    nc.compile()
    return nc


def _get_nc():
    if "nc" not in _CACHED:
        _CACHED["nc"] = build_kernel()
    return _CACHED["nc"]


def _bf16(x):
    return np.ascontiguousarray(
        np.asarray(x, np.float32).astype(ml_dtypes.bfloat16)
    )


def make_in_maps(image_features, text_features, Wq, bq, Wk, bk, Wv, bv):
    f8 = ml_dtypes.float8_e4m3
    Wq64 = np.asarray(Wq, np.float64)
    Wk64 = np.asarray(Wk, np.float64)
    m16 = _bf16(WS * (Wq64 @ Wk64.T))
    r = (Wk64 @ np.asarray(bq, np.float64)).astype(np.float32)
    wv16 = _bf16(Wv)
    img = np.asarray(image_features, np.float32)
    txt = np.asarray(text_features, np.float32)

    maps = []
    for b in range(B):
        txt8 = txt[b].astype(f8)  # [LK, TXT], quantized once from f32
        txtT8 = np.ascontiguousarray(
            txt8.T.reshape(NTP, 2, P, LK).transpose(0, 2, 1, 3)
            .reshape(NTP * P, 2 * LK)
        )
        maps.append({
            "imgT": np.ascontiguousarray(img[b].astype(ml_dtypes.bfloat16).T),
            "txt": _bf16(txt[b]),
            "txtT8": txtT8,
            "m16": m16,
            "wv": wv16,
            "ctxtr": (SCALE * (txt[b] @ r)).astype(np.float32),
        })
    return maps


def kernel(image_features, text_features, Wq, bq, Wk, bk, Wv, bv):
    in_maps = make_in_maps(image_features, text_features, Wq, bq, Wk, bk,
                           Wv, bv)
    res = run_bass_kernel_spmd(_get_nc(), in_maps, core_ids=list(range(B)))
    bv32 = np.asarray(bv, np.float32)
    return np.stack([res.results[b]["outT"].T + bv32 for b in range(B)])
